# revision 40
# baseline (speedup 1.0000x reference)
"""MixLoRA-MoE Trainium2 kernel: 8-core data-parallel over tokens.

Math restructuring vs the reference scan:
  final = sum_e w_e * (silu(g_e) * u_e) @ Wd.T + lora_down terms
        = hbar @ Wd.T + sum_e (wh_e @ Ad_e.T) @ (2 Bd_e.T)   [linearity]
  where wh_e = w_e * silu(g_e) * u_e and hbar = sum_e wh_e.
So the expensive down projection through Wd runs ONCE on the weighted
combination instead of once per expert.

On-chip layout is transposed ([feature_part, token_free]) so every matmul
operand streams from DRAM in its natural (pre-transposed on host) layout.
All heavy matmuls run in bf16 (f32 PSUM accumulation; f32r measures ~80ns
slower per matmul since FWL is disabled for fp32 weights). The router runs
on a separate exact-f32 path (top-2 selection flips on rounding near-ties).
Per-expert LoRA deltas are applied directly into the base PSUM accumulation
via a host-packed +/- pair tensor (expert e's matmul adds delta_e and
subtracts delta_{e-1}).
The per-expert lora-down (Ad) matmuls are deferred and issued back-to-back
as 32-column PE tiles so the array runs them concurrently; the Bd side is
host-packed into a single [E*R=128, D] stack so the down projection absorbs
it as one K=128 matmul per output tile.
"""
import os
import sys
import types

sys.path.insert(0, '/opt/trn_rl_repo')

import numpy as np
import ml_dtypes

BF16NP = ml_dtypes.bfloat16

# --- optional NTFF profiling shim (trace support under axon) ---
try:
    import antenv
    if 'antenv.axon_hooks' not in sys.modules:
        _m = types.ModuleType('antenv.axon_hooks')
        _hook_store = {}
        _m.set_axon_ntff_profile_hook = lambda h: _hook_store.__setitem__('h', h)
        _m.get_axon_ntff_profile_hook = lambda: _hook_store.get('h')
        sys.modules['antenv.axon_hooks'] = _m
        antenv.axon_hooks = _m
        try:
            from trn_agent_boot.trn_boot import _ntff_profile_via_ctypes
            _hook = _ntff_profile_via_ctypes('/opt/axon/libaxon_pjrt.so')
            if _hook is not None:
                _m.set_axon_ntff_profile_hook(_hook)
        except Exception:
            pass
except Exception:
    pass

import concourse.bass as bass
import concourse.mybir as mybir
from concourse import bacc
from concourse.tile import TileContext
from concourse import bass_utils

F32 = mybir.dt.float32
BF16 = mybir.dt.bfloat16
AF = mybir.ActivationFunctionType
ALU = mybir.AluOpType

P = 128
D = 2048          # d_model
F = 8192          # d_ff
E = 8             # experts
R = 16            # lora rank
NCORES = 8
T_FULL = 4096
TC = T_FULL // NCORES   # 512 tokens per core
DKT = D // P            # 16 k-tiles over d_model
FT = F // P             # 64 f-tiles over d_ff
DT_TILES = D // P       # 16 output d-tiles

LAST_RESULT = {}        # test harness introspection (exec_time_ns etc.)
_NC_CACHE = {}


def build_nc():
    if 'nc' in _NC_CACHE:
        return _NC_CACHE['nc']
    nc = bacc.Bacc(None, target_bir_lowering=False)

    # ---- DRAM I/O ----
    xt_d = nc.dram_tensor("xt", [D, TC], F32, kind="ExternalInput")
    wgt_d = nc.dram_tensor("wgt", [D, F], BF16, kind="ExternalInput")
    wut_d = nc.dram_tensor("wut", [D, F], BF16, kind="ExternalInput")
    wdt_d = nc.dram_tensor("wdt", [F, D], BF16, kind="ExternalInput")
    rwt_d = nc.dram_tensor("rwt", [D, E], F32, kind="ExternalInput")
    agp_d = nc.dram_tensor("agp", [D, E * R], BF16, kind="ExternalInput")
    aup_d = nc.dram_tensor("aup", [D, E * R], BF16, kind="ExternalInput")
    pmw_d = nc.dram_tensor("pmw", [64, E, F], BF16, kind="ExternalInput")
    adt_d = nc.dram_tensor("adt", [F, E, P], BF16, kind="ExternalInput")
    bd2_d = nc.dram_tensor("bd2", [E * R, D], BF16, kind="ExternalInput")
    oneh_d = nc.dram_tensor("oneh", [E, E, P], BF16, kind="ExternalInput")
    idt_d = nc.dram_tensor("idt", [P, P], F32, kind="ExternalInput")
    out_d = nc.dram_tensor("outT", [D, TC], F32, kind="ExternalOutput")

    with TileContext(nc) as tc:
        with tc.tile_pool(name="big", bufs=1) as big, \
             tc.tile_pool(name="wstream", bufs=4) as wstream, \
             tc.tile_pool(name="xstream", bufs=2) as xstream, \
             tc.tile_pool(name="pmstream", bufs=2) as pmstream, \
             tc.tile_pool(name="adtp", bufs=1) as adtp, \
             tc.tile_pool(name="ebuf", bufs=2) as ebuf, \
             tc.tile_pool(name="whbuf", bufs=6) as whbuf, \
             tc.tile_pool(name="gpsbuf", bufs=3) as gpsbuf, \
             tc.tile_pool(name="obuf", bufs=2) as obuf, \
             tc.tile_pool(name="ppg", bufs=3, space="PSUM") as ppg, \
             tc.tile_pool(name="ppu", bufs=2, space="PSUM") as ppu, \
             tc.tile_pool(name="ppt", bufs=1, space="PSUM") as ppt, \
             tc.tile_pool(name="ppk", bufs=1, space="PSUM") as ppk:

            # ---- persistent SBUF ----
            xtr = big.tile([P, DKT, TC], BF16, name="xtr")
            hbar = big.tile([P, FT, TC], BF16, name="hbar")
            wb = big.tile([P, E, TC], BF16, name="wb")
            spm = big.tile([64, E, TC], BF16, name="spm")
            psb = big.tile([P, TC], BF16, name="psb")
            bd2s = big.tile([P, D], BF16, name="bd2s")
            w8 = big.tile([E, 4, P], BF16, name="w8")
            rw = big.tile([P, DKT, E], F32, name="rw")
            oneh = big.tile([E, E, P], BF16, name="oneh")
            idt = big.tile([P, P], F32, name="idt")
            scr = big.tile([P, 16], F32, name="scr")
            wtl = big.tile([P, 4, E], F32, name="wtl")

            nc.sync.dma_start(rw, rwt_d.rearrange("(kt p) e -> p kt e", p=P))
            nc.sync.dma_start(idt, idt_d[:, :])

            # persistent PSUM: p accumulators (2 banks, 4 experts each via col groups)
            pbank = [ppk.tile([P, TC], F32, name=f"pbank{i}") for i in range(2)]

            # ================= router =================
            # x at full f32 lives in a short-lived pool: the router's top-2
            # selection needs exact f32 logits (bf16/f32r rounding flips
            # near-ties); the heavy matmuls below consume a bf16 copy.
            # Logits accumulate TRANSPOSED ([8 experts, 512 tokens]) with the
            # tiny rw slice stationary — an 8-column LDWEIGHTS instead of a
            # 128-column f32 one per matmul (f32 stationary disables FWL and
            # was costing 213ns per N=8 matmul).
            pslT = ppt.tile([P, TC], F32, name="trans")
            for tt in range(4):
                xt_tt = xstream.tile([P, DKT, P], F32, name="xchunk")
                nc.sync.dma_start(
                    xt_tt, xt_d[:, bass.ts(tt, P)].rearrange("(kt p) t -> p kt t", p=P))
                nc.vector.tensor_copy(xtr[:, :, bass.ts(tt, P)], xt_tt)
                for kt in range(DKT):
                    # full-f32 matmul: near-tie top-2 selection must match the
                    # reference's f32 logits
                    nc.tensor.matmul(pslT[0:E, bass.ts(tt, P)], rw[:, kt, :],
                                     xt_tt[:, kt, :], start=(kt == 0),
                                     stop=(kt == DKT - 1))
            lsT = big.tile([E, TC], F32, name="lsT")
            nc.vector.tensor_copy(lsT, pslT[0:E, :])
            for tt in range(4):
                # back to token-major [128 tokens, 8 experts] for the top-2
                # selection chain (free-axis reductions over experts)
                psl = ppt.tile([P, TC], F32, name="trans")
                nc.tensor.transpose(psl[:, 0:E], lsT[:, bass.ts(tt, P)],
                                    idt[0:E, 0:E])
                # top-2 selection happens on raw logits (exp is monotonic but the
                # ACT exp LUT has enough error to flip ~1e-4 near-ties)
                nmx = scr[:, 1:2]
                mx = scr[:, 0:1]
                m2 = scr[:, 2:3]
                rcp = scr[:, 3:4]
                z = scr[:, 4:12]
                lcp = wtl[:, 0, :]
                nc.vector.tensor_reduce(nmx, psl[:, 0:E], axis=mybir.AxisListType.X,
                                        op=ALU.max, negate=True)
                nc.vector.tensor_scalar_mul(mx, nmx, -1.0)
                nc.scalar.activation(z, psl[:, 0:E], AF.Exp, bias=nmx)
                # logits with the max knocked out: l - 1e30*(l >= max)
                lm1 = wtl[:, 1, :]
                nc.vector.tensor_scalar(lm1, psl[:, 0:E], mx, -1e30,
                                        op0=ALU.is_ge, op1=ALU.mult)
                nc.vector.tensor_tensor(lcp, psl[:, 0:E], lm1, op=ALU.add)
                nc.vector.tensor_reduce(m2, lcp, axis=mybir.AxisListType.X, op=ALU.max)
                # select z where logit >= second max; normalize by selected sum
                wsel = wtl[:, 1, :]
                nc.vector.scalar_tensor_tensor(wsel, psl[:, 0:E], m2, z,
                                               op0=ALU.is_ge, op1=ALU.mult)
                nc.vector.tensor_reduce(rcp, wsel, axis=mybir.AxisListType.X, op=ALU.add)
                nc.vector.reciprocal(rcp, rcp)
                wcur = wtl[:, 2 + (tt % 2), :]
                nc.vector.tensor_scalar_mul(wcur, wsel, rcp)
                # transpose [128t, 8e] -> psum [8e, 128t]
                psw = ppt.tile([P, TC], F32, name="trans")
                nc.tensor.transpose(psw[0:E, 0:P], wcur, idt)
                nc.vector.tensor_copy(w8[:, tt, :], psw[0:E, 0:P])
            # broadcast rows of w8 -> wb[128, e, TC] via one-hot matmuls
            nc.sync.dma_start(oneh, oneh_d[:, :, :])
            w8flat = w8.rearrange("p a b -> p (a b)")
            for e in range(E):
                pswb = ppt.tile([P, TC], F32, name="trans")
                nc.tensor.matmul(pswb, oneh[:, e, :], w8flat, start=True, stop=True)
                nc.vector.tensor_copy(wb[:, e, :], pswb)

            # ================= lora-A projections (s) =================
            for gi, src in enumerate((agp_d, aup_d)):
                ap_t = xstream.tile([P, DKT, P], BF16, name="apchunk")
                nc.sync.dma_start(ap_t, src.rearrange("(kt p) m -> p kt m", p=P))
                sps = ppu.tile([P, TC], F32, name="banku")
                for kt in range(DKT):
                    nc.tensor.matmul(sps, ap_t[:, kt, :], xtr[:, kt, :],
                                     start=(kt == 0), stop=(kt == DKT - 1))
                s_stage = ebuf.tile([P, TC], BF16, name="t1")
                nc.vector.tensor_copy(s_stage, sps)
                # scatter into +/- pair layout rows: [prev(16) | cur(16)] per expert
                base = 32 * gi
                nc.gpsimd.memset(spm[base:base + R, 0, :], 0.0)
                for e in range(E):
                    if e >= 1:
                        nc.sync.dma_start(spm[base:base + R, e, :],
                                          s_stage[(e - 1) * R:e * R, :])
                    nc.sync.dma_start(spm[base + R:base + 2 * R, e, :],
                                      s_stage[e * R:(e + 1) * R, :])

            # ================= main f-loop =================
            for f in range(FT):
                wg_t = wstream.tile([P, DKT, P], BF16, name="wchunk")
                nc.sync.dma_start(wg_t, wgt_d[:, bass.ts(f, P)].rearrange("(kt p) m -> p kt m", p=P))
                wu_t = wstream.tile([P, DKT, P], BF16, name="wchunk")
                nc.sync.dma_start(wu_t, wut_d[:, bass.ts(f, P)].rearrange("(kt p) m -> p kt m", p=P))
                pm_t = pmstream.tile([64, E, P], BF16, name="pmt")
                nc.sync.dma_start(pm_t, pmw_d[:, :, bass.ts(f, P)])
                if f % 8 == 0:
                    adt_t = adtp.tile([P, 8, E, P], BF16, name="adt")
                    nc.sync.dma_start(
                        adt_t, adt_d[f * P:(f + 8) * P, :, :].rearrange(
                            "(fo p) e r -> p fo e r", p=P))

                bank_g = ppg.tile([P, TC], F32, name="bankg")
                bank_u = ppu.tile([P, TC], F32, name="banku")
                for kt in range(DKT):
                    nc.tensor.matmul(bank_g, wg_t[:, kt, :], xtr[:, kt, :],
                                     start=(kt == 0), stop=False)
                # first gate delta right after the gate block so the expert
                # chain (silu onward) starts while the up block still streams
                nc.tensor.matmul(bank_g, pm_t[0:32, 0, :], spm[0:32, 0, :],
                                 start=False, stop=False)
                for kt in range(DKT):
                    nc.tensor.matmul(bank_u, wu_t[:, kt, :], xtr[:, kt, :],
                                     start=(kt == 0), stop=False)
                nc.tensor.matmul(bank_u, pm_t[32:64, 0, :], spm[32:64, 0, :],
                                 start=False, stop=False)

                whw = []
                for e in range(E):
                    s_act = ebuf.tile([P, TC], BF16, name="sact")
                    nc.scalar.activation(s_act, bank_g, AF.Silu)
                    # t1 = h_e (unweighted): the lora-down matmul consumes it
                    # directly; the router weight w_e commutes out of that
                    # contraction (applied at extraction instead)
                    t1 = whbuf.tile([P, TC], BF16, name="wh")
                    nc.vector.scalar_tensor_tensor(t1, bank_u, 1.0, s_act,
                                                   op0=ALU.bypass, op1=ALU.mult)
                    # next expert's +/- deltas, emitted adjacently: a K=32
                    # pair issued back-to-back costs ~320ns total and only
                    # one row-group LDW conflict on the following K=128 MM
                    if e + 1 < E:
                        nc.tensor.matmul(bank_g, pm_t[0:32, e + 1, :],
                                         spm[0:32, e + 1, :],
                                         start=False, stop=(e + 1 == E - 1))
                        nc.tensor.matmul(bank_u, pm_t[32:64, e + 1, :],
                                         spm[32:64, e + 1, :],
                                         start=False, stop=(e + 1 == E - 1))
                    # lora-down A: p_e += adt_e.T @ h_e. The per-expert Ad
                    # weights are zero-padded to the full 128 output columns
                    # (expert's rows at offset 32*(e%4)), so this is a plain
                    # full-array matmul writing the whole bank — no PE
                    # tiling-mode switches.
                    nc.tensor.matmul(pbank[e // 4],
                                     adt_t[:, f % 8, e, :], t1,
                                     start=(f == 0 and e % 4 == 0),
                                     stop=(f == FT - 1 and e % 4 == 3))
                    # weighted copy feeds only the hbar accumulation tree
                    # (DVE mult; the adds run on gpsimd off the critical path)
                    wv = gpsbuf.tile([P, TC], BF16, name="whw")
                    nc.vector.tensor_tensor(wv, t1, wb[:, e, :], op=ALU.mult)
                    whw.append(wv)
                    if e == 1:
                        nc.gpsimd.tensor_tensor(hbar[:, f, :], whw[0], whw[1], op=ALU.add)
                    elif e > 1:
                        nc.gpsimd.tensor_tensor(hbar[:, f, :], hbar[:, f, :], wv, op=ALU.add)

            # extract p to sbuf with the router weight applied per expert-row
            # group (wb's first 32 partitions broadcast w_e), stacked
            # [E*R=128, TC] (expert-major)
            for b in range(2):
                p_stage = ebuf.tile([P, TC], BF16, name="t1")
                for eo in range(4):
                    e = b * 4 + eo
                    nc.vector.scalar_tensor_tensor(
                        p_stage[32 * eo:32 * eo + R, :],
                        pbank[b][32 * eo:32 * eo + R, :], 1.0,
                        wb[0:R, e, :], op0=ALU.bypass, op1=ALU.mult)
                    nc.sync.dma_start(psb[e * R:(e + 1) * R, :],
                                      p_stage[32 * eo:32 * eo + R, :])

            # ================= down projection =================
            nc.sync.dma_start(bd2s, bd2_d[:, :])
            for d in range(DT_TILES):
                psd = ppg.tile([P, TC], F32, name="bankg")
                for fc in range(4):
                    wd_t = wstream.tile([P, DKT, P], BF16, name="wdchunk")
                    nc.sync.dma_start(
                        wd_t, wdt_d[fc * 2048:(fc + 1) * 2048, bass.ts(d, P)].rearrange(
                            "(kt p) m -> p kt m", p=P))
                    for kt in range(DKT):
                        nc.tensor.matmul(psd, wd_t[:, kt, :], hbar[:, fc * DKT + kt, :],
                                         start=(fc == 0 and kt == 0), stop=False)
                # stacked lora-down B: one K=128 matmul absorbs all 8 experts
                nc.tensor.matmul(psd, bd2s[:, bass.ts(d, P)], psb,
                                 start=False, stop=True)
                o_t = obuf.tile([P, TC], F32, name="osb")
                nc.scalar.activation(o_t, psd, AF.Copy)
                nc.sync.dma_start(out_d[bass.ts(d, P), :], o_t)

    nc.finalize()
    _NC_CACHE['nc'] = nc
    return nc


def _host_prep(hidden_states, router_w, Wg, Wu, Wd, Ag, Bg, Au, Bu, Ad, Bd):
    f32 = np.float32
    X = np.ascontiguousarray(hidden_states.reshape(T_FULL, D), dtype=f32)
    xT = np.ascontiguousarray(X.T)
    shared = {
        "wgt": np.ascontiguousarray(Wg.T).astype(BF16NP),
        "wut": np.ascontiguousarray(Wu.T).astype(BF16NP),
        "wdt": np.ascontiguousarray(Wd.T).astype(BF16NP),
        "rwt": np.ascontiguousarray(router_w.T, dtype=f32),
        "agp": np.ascontiguousarray(Ag.transpose(2, 0, 1).reshape(D, E * R)).astype(BF16NP),
        "aup": np.ascontiguousarray(Au.transpose(2, 0, 1).reshape(D, E * R)).astype(BF16NP),
    }
    # +/- pair tensor: rows 0:16 gate-prev(-), 16:32 gate-cur(+), 32:48 up-prev(-), 48:64 up-cur(+)
    pmw = np.zeros((64, E, F), dtype=f32)
    BgT = np.transpose(Bg, (0, 2, 1))  # [E, R, F]
    BuT = np.transpose(Bu, (0, 2, 1))
    for e in range(E):
        if e >= 1:
            pmw[0:R, e] = -2.0 * BgT[e - 1]
            pmw[32:48, e] = -2.0 * BuT[e - 1]
        pmw[R:32, e] = 2.0 * BgT[e]
        pmw[48:64, e] = 2.0 * BuT[e]
    shared["pmw"] = pmw.astype(BF16NP)
    # zero-padded to full 128 columns: expert e's rank rows sit at column
    # offset 32*(e%4) (its row group in the psum accumulator bank e//4)
    adt = np.zeros((F, E, P), dtype=f32)
    AdT = Ad.transpose(2, 0, 1)  # [F, E, R]
    for e in range(E):
        adt[:, e, 32 * (e % 4):32 * (e % 4) + R] = AdT[:, e, :]
    shared["adt"] = adt.astype(BF16NP)
    # stacked on expert-major rows: [E*R, D]
    shared["bd2"] = np.ascontiguousarray(
        (2.0 * Bd.transpose(0, 2, 1)).reshape(E * R, D)).astype(BF16NP)
    oneh = np.zeros((E, E, P), dtype=f32)
    for e in range(E):
        oneh[e, e, :] = 1.0
    shared["oneh"] = oneh.astype(BF16NP)
    shared["idt"] = np.eye(P, dtype=f32)
    in_maps = []
    for c in range(NCORES):
        m = dict(shared)
        m["xt"] = np.ascontiguousarray(xT[:, c * TC:(c + 1) * TC])
        in_maps.append(m)
    return in_maps


def kernel(hidden_states, router_w, Wg, Wu, Wd, Ag, Bg, Au, Bu, Ad, Bd):
    hidden_states = np.asarray(hidden_states)
    nc = build_nc()
    in_maps = _host_prep(np.asarray(hidden_states, dtype=np.float32),
                         np.asarray(router_w), np.asarray(Wg), np.asarray(Wu),
                         np.asarray(Wd), np.asarray(Ag), np.asarray(Bg),
                         np.asarray(Au), np.asarray(Bu), np.asarray(Ad),
                         np.asarray(Bd))
    trace = bool(os.environ.get("TRNK_TRACE"))
    res = bass_utils.run_bass_kernel_spmd(
        nc, in_maps, core_ids=list(range(NCORES)), trace=trace)
    LAST_RESULT['exec_time_ns'] = res.exec_time_ns
    LAST_RESULT['res'] = res
    out = np.empty((T_FULL, D), dtype=np.float32)
    for c in range(NCORES):
        out[c * TC:(c + 1) * TC, :] = res.results[c]["outT"].T
    return out.reshape(hidden_states.shape[0], hidden_states.shape[1], D)
